# revision 3
# baseline (speedup 1.0000x reference)
"""2-layer GAT (PyG GATConv, heads=1) on 8 Trainium2 NeuronCores.

Strategy (dst-owner sharding, per spec sharding_hint):
  - Nodes split into 8 contiguous chunks of N/8; edges owned by dst's core.
  - 3 NEFF launches (host does only data movement between them):
    NEFF#1: per-core h1 = embed_chunk @ W1, s1/d1 = h1 @ a_{src,dst}1,
            emitted as bf16 hi/lo split rows -> host concats into a full
            gather table T1 [N, 384]bf16 = [h_hi|h_lo|1.0|s_hi|s_lo|pad].
    NEFF#2: L1 edge phase per core: dma_gather T1 rows by edge src,
            attention weights w_e = exp(LeakyReLU(s_src + d_dst)) computed
            via the separable form  w = max(exp(s)exp(d), exp(.2s)exp(.2d)),
            folded into a per-128-edge-group indicator matrix
            S_alpha[e, c] = w_e * 1[dstcol_e == c]  (one fused DVE op),
            aggregated on the TensorEngine: psum += S_alpha^T @ [h|1].
            The trailing ones column yields the softmax denominator Z free.
            Tail: x2 = relu(out1+b1); h2 = x2 @ W2; s2/d2 -> T2 chunks.
    NEFF#3: same edge machinery on T2 [N, 256]bf16, final sigmoid.
  - Edges are bucketed into <=32768-row source "sets" (dma_gather idx is
    int16) and into 127-dst psum windows; group counts G[set][window] are
    maxed across cores so all 8 cores run one SPMD instruction stream.
"""
import sys

if '/opt/trn_rl_repo' not in sys.path:
    sys.path.insert(0, '/opt/trn_rl_repo')

import numpy as np
import ml_dtypes

from concourse import bacc, mybir
import concourse.tile as tile
from concourse.bass_utils import run_bass_kernel_spmd
from concourse.masks import make_identity

BF16 = ml_dtypes.bfloat16
NCORES = 8
RESULTS = []  # BassKernelResults per NEFF launch (for test harness introspection)
WIN = 127          # dsts per psum window (col 127 = dummy slot)
MW = 2             # windows per gather megatile
SETROWS = 32768    # int16 gather index range
F32 = mybir.dt.float32
BF = mybir.dt.bfloat16
I16 = mybir.dt.int16
AF = mybir.ActivationFunctionType
OP = mybir.AluOpType


# ----------------------------------------------------------------- host pre
def _preprocess(edge_index, N):
    CH = N // NCORES
    NW = -(-CH // WIN)
    NS = -(-N // SETROWS)
    src = np.concatenate([edge_index[0], np.arange(N, dtype=np.int64)])
    dst = np.concatenate([edge_index[1], np.arange(N, dtype=np.int64)])
    src = src.astype(np.int64)
    dst = dst.astype(np.int64)
    owner = dst // CH
    dl = dst - owner * CH
    sid = src // SETROWS

    cnt = np.zeros((NCORES, NS, NW), np.int64)
    percs = []
    for c in range(NCORES):
        mc = owner == c
        percs.append((src[mc], dl[mc], sid[mc]))
        for s in range(NS):
            ms = percs[c][2] == s
            w = percs[c][1][ms] // WIN
            cnt[c, s] = np.bincount(w, minlength=NW)
    G = -(-cnt.max(axis=0) // 128)          # [NS, NW] groups per (set, window)
    G[cnt.max(axis=0) == 0] = 0

    cumG = np.zeros((NS, NW + 1), np.int64)
    cumG[:, 1:] = np.cumsum(G, axis=1)
    nslot = 128 * cumG[:, -1]               # per-set stream length

    cores = []
    for c in range(NCORES):
        csrc, cdl, csid = percs[c]
        gidx, dcol = [], []
        for s in range(NS):
            ms = csid == s
            esrc, edl = csrc[ms], cdl[ms]
            order = np.argsort(edl, kind='stable')
            esrc, edl = esrc[order], edl[order]
            w = edl // WIN
            col = edl - w * WIN
            # rank within window
            cc = np.zeros(NW + 1, np.int64)
            cc[1:] = np.cumsum(np.bincount(w, minlength=NW))
            rank = np.arange(len(edl)) - cc[w]
            slot = 128 * cumG[s][w] + rank
            arr_i = np.zeros(nslot[s], np.int16)
            arr_c = np.full(nslot[s], 127.0, np.float32)
            arr_i[slot] = (esrc - s * SETROWS).astype(np.int16)
            arr_c[slot] = col
            gi = np.tile(arr_i.reshape(-1, 16).T, (8, 1)) if nslot[s] else \
                np.zeros((128, 0), np.int16)
            dc = arr_c.reshape(-1, 128).T.astype(BF16) if nslot[s] else \
                np.zeros((128, 0), BF16)
            gidx.append(np.ascontiguousarray(gi))
            dcol.append(np.ascontiguousarray(dc))
        cores.append((gidx, dcol))
    return dict(CH=CH, NW=NW, NS=NS, G=G, cumG=cumG, nslot=nslot, cores=cores)


# ------------------------------------------------------------------ NEFF #1
def _build_neff1(N, C, H, CH):
    nc = bacc.Bacc(None, target_bir_lowering=False)
    xT = nc.declare_dram_parameter("xT", [C, CH], F32, isOutput=False)
    W1 = nc.declare_dram_parameter("W1", [C, H], F32, isOutput=False)
    a1s = nc.declare_dram_parameter("a1s", [H, 1], F32, isOutput=False)
    a1d = nc.declare_dram_parameter("a1d", [H, 1], F32, isOutput=False)
    hhi = nc.declare_dram_parameter("hhi", [H, CH], BF, isOutput=True)
    hlo = nc.declare_dram_parameter("hlo", [H, CH], BF, isOutput=True)
    shi = nc.declare_dram_parameter("shi", [1, CH], BF, isOutput=True)
    slo = nc.declare_dram_parameter("slo", [1, CH], BF, isOutput=True)
    d1o = nc.declare_dram_parameter("d1o", [1, CH], F32, isOutput=True)

    KT = -(-C // 128)
    with tile.TileContext(nc) as tc:
        with tc.tile_pool(name="cst", bufs=1) as cp, \
             tc.tile_pool(name="wk", bufs=3) as wp, \
             tc.tile_pool(name="ps", bufs=2, space="PSUM") as pp, \
             tc.tile_pool(name="ps1", bufs=2, space="PSUM") as pp1:
            xts, w1s = [], []
            for k in range(KT):
                kc = min(128, C - 128 * k)
                xt = cp.tile([kc, CH], F32, tag=f"xt{k}")
                nc.sync.dma_start(out=xt[:], in_=xT[128 * k:128 * k + kc, :])
                w1 = cp.tile([kc, H], F32, tag=f"w1{k}")
                nc.sync.dma_start(out=w1[:], in_=W1[128 * k:128 * k + kc, :])
                xts.append(xt)
                w1s.append(w1)
            asb = cp.tile([H, 1], F32, tag="a1s")
            nc.sync.dma_start(out=asb[:], in_=a1s[:])
            adb = cp.tile([H, 1], F32, tag="a1d")
            nc.sync.dma_start(out=adb[:], in_=a1d[:])
            h1T = cp.tile([H, CH], F32, tag="h1T")

            CW = 500
            for o in range(0, CH, CW):
                cw = min(CW, CH - o)
                ph = pp.tile([H, CW], F32, space="PSUM", tag="ph")
                for k in range(KT):
                    nc.tensor.matmul(out=ph[:, :cw], lhsT=w1s[k][:],
                                     rhs=xts[k][:, o:o + cw],
                                     start=(k == 0), stop=(k == KT - 1))
                nc.vector.tensor_copy(out=h1T[:, o:o + cw], in_=ph[:, :cw])
                hh = wp.tile([H, CW], BF, tag="hh")
                nc.scalar.activation(hh[:, :cw], ph[:, :cw], AF.Copy)
                tmp = wp.tile([H, CW], F32, tag="tmp")
                nc.vector.tensor_tensor(out=tmp[:, :cw], in0=ph[:, :cw],
                                        in1=hh[:, :cw], op=OP.subtract)
                hl = wp.tile([H, CW], BF, tag="hl")
                nc.vector.tensor_copy(out=hl[:, :cw], in_=tmp[:, :cw])
                nc.sync.dma_start(out=hhi[:, o:o + cw], in_=hh[:, :cw])
                nc.sync.dma_start(out=hlo[:, o:o + cw], in_=hl[:, :cw])
            for o in range(0, CH, CW):
                cw = min(CW, CH - o)
                ps = pp1.tile([1, CW], F32, space="PSUM", tag="psv")
                nc.tensor.matmul(out=ps[:, :cw], lhsT=asb[:],
                                 rhs=h1T[:, o:o + cw], start=True, stop=True)
                sh = wp.tile([1, CW], BF, tag="sh")
                nc.scalar.activation(sh[:, :cw], ps[:, :cw], AF.Copy)
                tmp = wp.tile([1, CW], F32, tag="tms")
                nc.vector.tensor_tensor(out=tmp[:, :cw], in0=ps[:, :cw],
                                        in1=sh[:, :cw], op=OP.subtract)
                sl = wp.tile([1, CW], BF, tag="sl")
                nc.vector.tensor_copy(out=sl[:, :cw], in_=tmp[:, :cw])
                nc.sync.dma_start(out=shi[:, o:o + cw], in_=sh[:, :cw])
                nc.sync.dma_start(out=slo[:, o:o + cw], in_=sl[:, :cw])
                pd = pp1.tile([1, CW], F32, space="PSUM", tag="pdv")
                nc.tensor.matmul(out=pd[:, :cw], lhsT=adb[:],
                                 rhs=h1T[:, o:o + cw], start=True, stop=True)
                dv = wp.tile([1, CW], F32, tag="dv")
                nc.vector.tensor_copy(out=dv[:, :cw], in_=pd[:, :cw])
                nc.sync.dma_start(out=d1o[:, o:o + cw], in_=dv[:, :cw])
    nc.finalize()
    return nc


# --------------------------------------------------------- edge-phase NEFFs
def _build_edge_neff(N, CH, NW, NS, G, cumG, nslot, layer, FH, FO, Hnext):
    """layer 1: aggregates FH-dim messages, computes x2=relu(.+b1), h2/s2/d2.
       layer 2: aggregates FH-dim messages, emits sigmoid output [CH, FH].
       FH: feature dim of this layer's h.  FO: next-layer dim (layer 1 only).
    """
    TC = 384 if layer == 1 else 256
    SC = 2 * FH + 1                     # s_hi col (after h_hi, h_lo, ones)
    RC = 2 * FH + 1                     # rhs cols: h_hi | h_lo | ones
    WT = NW * WIN
    BW = WT + 128                       # padded width for B/D slices

    nc = bacc.Bacc(None, target_bir_lowering=False)
    T = nc.declare_dram_parameter("T", [N, TC], BF, isOutput=False)
    dloc = nc.declare_dram_parameter("dloc", [1, BW], F32, isOutput=False)
    iot = nc.declare_dram_parameter("iot", [128, 128], BF, isOutput=False)
    one1 = nc.declare_dram_parameter("one1", [1, 128], BF, isOutput=False)
    brep = nc.declare_dram_parameter("brep", [128, FH], F32, isOutput=False)
    gidx_d, dcol_d = [], []
    for s in range(NS):
        if nslot[s] == 0:
            gidx_d.append(None)
            dcol_d.append(None)
            continue
        gidx_d.append(nc.declare_dram_parameter(
            f"gidx{s}", [128, nslot[s] // 16], I16, isOutput=False))
        dcol_d.append(nc.declare_dram_parameter(
            f"dcol{s}", [128, nslot[s] // 128], BF, isOutput=False))
    if layer == 1:
        W2 = nc.declare_dram_parameter("W2", [FH, FO], F32, isOutput=False)
        a2s = nc.declare_dram_parameter("a2s", [FO, 1], F32, isOutput=False)
        a2d = nc.declare_dram_parameter("a2d", [FO, 1], F32, isOutput=False)
        hhi = nc.declare_dram_parameter("hhi", [FO, WT], BF, isOutput=True)
        hlo = nc.declare_dram_parameter("hlo", [FO, WT], BF, isOutput=True)
        shi = nc.declare_dram_parameter("shi", [1, WT], BF, isOutput=True)
        slo = nc.declare_dram_parameter("slo", [1, WT], BF, isOutput=True)
        d2o = nc.declare_dram_parameter("d2o", [1, WT], F32, isOutput=True)
    else:
        outp = nc.declare_dram_parameter("out", [CH, FH], F32, isOutput=True)

    # megatile group spans per set
    mts = []
    for wa in range(0, NW, MW):
        wb = min(wa + MW, NW)
        span = [(int(cumG[s][wa]), int(cumG[s][wb])) for s in range(NS)]
        mts.append((wa, wb, span))
    maxg = [max((b - a) for _, _, sp in mts for (a, b) in [sp[s]]) or 1
            for s in range(NS)]

    with tile.TileContext(nc) as tc:
        with tc.tile_pool(name="cst", bufs=1) as cp:
            iosb = cp.tile([128, 128], BF, tag="io")
            nc.sync.dma_start(out=iosb[:], in_=iot[:])
            onsb = cp.tile([1, 128], BF, tag="on")
            nc.sync.dma_start(out=onsb[:], in_=one1[:])
            bsb = cp.tile([128, FH], F32, tag="bs")
            nc.sync.dma_start(out=bsb[:], in_=brep[:])
            Bt = cp.tile([1, BW], BF, tag="Bt")
            Dt = cp.tile([1, BW], BF, tag="Dt")
            with tc.tile_pool(name="dtmp", bufs=1) as dtp:
                dsb = dtp.tile([1, BW], F32, tag="ds")
                nc.sync.dma_start(out=dsb[:], in_=dloc[:])
                nc.scalar.activation(Bt[:], dsb[:], AF.Exp)
                nc.scalar.activation(Dt[:], dsb[:], AF.Exp, scale=0.2)
            if layer == 1:
                idn = cp.tile([128, 128], F32, tag="idn")
                make_identity(nc, idn[:])
                x2T = cp.tile([128, WT], F32, tag="x2T")
                w2sb = cp.tile([FH, FO], F32, tag="w2")
                nc.sync.dma_start(out=w2sb[:], in_=W2[:])
                a2ssb = cp.tile([FO, 1], F32, tag="a2s")
                nc.sync.dma_start(out=a2ssb[:], in_=a2s[:])
                a2dsb = cp.tile([FO, 1], F32, tag="a2d")
                nc.sync.dma_start(out=a2dsb[:], in_=a2d[:])

            with tc.tile_pool(name="gth", bufs=2) as gp, \
                 tc.tile_pool(name="wk", bufs=4) as wp, \
                 tc.tile_pool(name="msk", bufs=4) as mp, \
                 tc.tile_pool(name="pm", bufs=2, space="PSUM") as pmp, \
                 tc.tile_pool(name="pb", bufs=2, space="PSUM") as pbp, \
                 tc.tile_pool(name="pt", bufs=2, space="PSUM") as ptp:
                for wa, wb, span in mts:
                    gts, Ats, Cts, dcs = [], [], [], []
                    for s in range(NS):
                        ga, gb = span[s]
                        if gb == ga:
                            gts.append(None)
                            Ats.append(None)
                            Cts.append(None)
                            dcs.append(None)
                            continue
                        gsp = gb - ga
                        ix = gp.tile([128, maxg[s] * 8], I16, tag=f"ix{s}")
                        nc.sync.dma_start(out=ix[:, :gsp * 8],
                                          in_=gidx_d[s][:, ga * 8:gb * 8])
                        gt = gp.tile([128, maxg[s], TC], BF, tag=f"gt{s}")
                        nc.gpsimd.dma_gather(
                            out_ap=gt[:, :gsp, :],
                            in_ap=T[s * SETROWS:, :],
                            idxs_ap=ix[:, :gsp * 8],
                            num_idxs=gsp * 128,
                            num_idxs_reg=gsp * 128,
                            elem_size=TC,
                            single_packet=False,
                        )
                        dc = wp.tile([128, maxg[s]], BF, tag=f"dc{s}")
                        nc.sync.dma_start(out=dc[:, :gsp],
                                          in_=dcol_d[s][:, ga:gb])
                        se = wp.tile([128, maxg[s]], F32, tag=f"se{s}")
                        nc.vector.tensor_tensor(out=se[:, :gsp],
                                                in0=gt[:, :gsp, SC],
                                                in1=gt[:, :gsp, SC + 1],
                                                op=OP.add)
                        At = wp.tile([128, maxg[s]], F32, tag=f"At{s}")
                        nc.scalar.activation(At[:, :gsp], se[:, :gsp], AF.Exp)
                        Ct = wp.tile([128, maxg[s]], F32, tag=f"Ct{s}")
                        nc.scalar.activation(Ct[:, :gsp], se[:, :gsp], AF.Exp,
                                             scale=0.2)
                        gts.append(gt)
                        Ats.append(At)
                        Cts.append(Ct)
                        dcs.append(dc)
                    for w in range(wa, wb):
                        ngrp = int(G[:, w].sum())
                        if ngrp == 0:
                            continue
                        w0 = w * WIN
                        pb = pbp.tile([128, 128], F32, space="PSUM", tag="pb")
                        nc.tensor.matmul(out=pb[:], lhsT=onsb[:],
                                         rhs=Bt[:, w0:w0 + 128],
                                         start=True, stop=True)
                        Br = mp.tile([128, 128], BF, tag="Br")
                        nc.vector.tensor_copy(out=Br[:], in_=pb[:])
                        pd2 = pbp.tile([128, 128], F32, space="PSUM", tag="pd2")
                        nc.tensor.matmul(out=pd2[:], lhsT=onsb[:],
                                         rhs=Dt[:, w0:w0 + 128],
                                         start=True, stop=True)
                        Dr = mp.tile([128, 128], BF, tag="Dr")
                        nc.vector.tensor_copy(out=Dr[:], in_=pd2[:])

                        psum = pmp.tile([128, RC], F32, space="PSUM", tag="ps")
                        gi = 0
                        for s in range(NS):
                            ga, _ = span[s]
                            for j in range(int(G[s][w])):
                                g = int(cumG[s][w]) - ga + j
                                gg = g
                                t2 = mp.tile([128, 128], BF, tag="t2")
                                nc.scalar.activation(
                                    t2[:], Dr[:], AF.Copy,
                                    scale=Cts[s][:, gg:gg + 1])
                                t1 = mp.tile([128, 128], BF, tag="t1")
                                nc.vector.scalar_tensor_tensor(
                                    out=t1[:], in0=Br[:],
                                    scalar=Ats[s][:, gg:gg + 1], in1=t2[:],
                                    op0=OP.mult, op1=OP.max)
                                sal = mp.tile([128, 128], BF, tag="sal")
                                nc.vector.scalar_tensor_tensor(
                                    out=sal[:], in0=iosb[:],
                                    scalar=dcs[s][:, gg:gg + 1], in1=t1[:],
                                    op0=OP.is_equal, op1=OP.mult)
                                nc.tensor.matmul(
                                    out=psum[:], lhsT=sal[:],
                                    rhs=gts[s][:, g, 0:RC],
                                    start=(gi == 0), stop=(gi == ngrp - 1))
                                gi += 1
                        # ---- window tail
                        pc = wp.tile([128, RC], F32, tag="pc")
                        nc.vector.tensor_copy(out=pc[:], in_=psum[:])
                        u = wp.tile([128, FH], F32, tag="u")
                        nc.vector.tensor_tensor(out=u[:], in0=pc[:, 0:FH],
                                                in1=pc[:, FH:2 * FH],
                                                op=OP.add)
                        zeps = wp.tile([128, 1], F32, tag="zeps")
                        nc.vector.tensor_scalar(
                            out=zeps[:], in0=pc[:, 2 * FH:2 * FH + 1],
                            scalar1=1e-16, scalar2=None, op0=OP.add)
                        rz = wp.tile([128, 1], F32, tag="rz")
                        nc.vector.reciprocal(out=rz[:], in_=zeps[:])
                        o1 = wp.tile([128, FH], F32, tag="o1")
                        nc.vector.tensor_scalar(
                            out=o1[:], in0=u[:], scalar1=rz[:], scalar2=None,
                            op0=OP.mult)
                        xb = wp.tile([128, FH], F32, tag="xb")
                        nc.vector.tensor_tensor(out=xb[:], in0=o1[:],
                                                in1=bsb[:], op=OP.add)
                        nr = min(WIN, CH - w0)
                        if layer == 1:
                            x2 = wp.tile([128, FH], F32, tag="x2")
                            nc.vector.tensor_scalar(
                                out=x2[:], in0=xb[:], scalar1=0.0,
                                scalar2=None, op0=OP.max)
                            pt = ptp.tile([128, 128], F32, space="PSUM",
                                          tag="pt")
                            nc.tensor.transpose(pt[:], x2[:], idn[:])
                            nc.vector.tensor_copy(out=x2T[:, w0:w0 + WIN],
                                                  in_=pt[:, 0:WIN])
                        else:
                            sg = wp.tile([128, FH], F32, tag="sg")
                            nc.scalar.activation(sg[:], xb[:], AF.Sigmoid)
                            nc.sync.dma_start(out=outp[w0:w0 + nr, :],
                                              in_=sg[0:nr, :])

            if layer == 1:
                with tc.tile_pool(name="tl", bufs=3) as tp, \
                     tc.tile_pool(name="tc1", bufs=1) as tcp, \
                     tc.tile_pool(name="ph2", bufs=2, space="PSUM") as php, \
                     tc.tile_pool(name="psv", bufs=2, space="PSUM") as psp:
                    h2T = tcp.tile([FO, WT], F32, tag="h2T")
                    CW = 512
                    for o in range(0, WT, CW):
                        cw = min(CW, WT - o)
                        ph = php.tile([FO, CW], F32, space="PSUM", tag="ph")
                        nc.tensor.matmul(out=ph[:, :cw], lhsT=w2sb[:],
                                         rhs=x2T[:, o:o + cw],
                                         start=True, stop=True)
                        nc.vector.tensor_copy(out=h2T[:, o:o + cw], in_=ph[:, :cw])
                        hh = tp.tile([FO, CW], BF, tag="hh")
                        nc.scalar.activation(hh[:, :cw], ph[:, :cw], AF.Copy)
                        tmp = tp.tile([FO, CW], F32, tag="tmp")
                        nc.vector.tensor_tensor(out=tmp[:, :cw], in0=ph[:, :cw],
                                                in1=hh[:, :cw], op=OP.subtract)
                        hl = tp.tile([FO, CW], BF, tag="hl")
                        nc.vector.tensor_copy(out=hl[:, :cw], in_=tmp[:, :cw])
                        nc.sync.dma_start(out=hhi[:, o:o + cw], in_=hh[:, :cw])
                        nc.sync.dma_start(out=hlo[:, o:o + cw], in_=hl[:, :cw])
                    for o in range(0, WT, CW):
                        cw = min(CW, WT - o)
                        ps = psp.tile([1, CW], F32, space="PSUM", tag="ps2")
                        nc.tensor.matmul(out=ps[:, :cw], lhsT=a2ssb[:],
                                         rhs=h2T[:, o:o + cw],
                                         start=True, stop=True)
                        sh = tp.tile([1, CW], BF, tag="sh")
                        nc.scalar.activation(sh[:, :cw], ps[:, :cw], AF.Copy)
                        tmp = tp.tile([1, CW], F32, tag="tms")
                        nc.vector.tensor_tensor(out=tmp[:, :cw], in0=ps[:, :cw],
                                                in1=sh[:, :cw], op=OP.subtract)
                        sl = tp.tile([1, CW], BF, tag="sl")
                        nc.vector.tensor_copy(out=sl[:, :cw], in_=tmp[:, :cw])
                        nc.sync.dma_start(out=shi[:, o:o + cw], in_=sh[:, :cw])
                        nc.sync.dma_start(out=slo[:, o:o + cw], in_=sl[:, :cw])
                        pd = psp.tile([1, CW], F32, space="PSUM", tag="pd")
                        nc.tensor.matmul(out=pd[:, :cw], lhsT=a2dsb[:],
                                         rhs=h2T[:, o:o + cw],
                                         start=True, stop=True)
                        dv = tp.tile([1, CW], F32, tag="dv")
                        nc.vector.tensor_copy(out=dv[:, :cw], in_=pd[:, :cw])
                        nc.sync.dma_start(out=d2o[:, o:o + cw], in_=dv[:, :cw])
    nc.finalize()
    return nc


# ------------------------------------------------------------------- driver
def kernel(edge_index, embed, W1, a_src1, a_dst1, b1, W2, a_src2, a_dst2, b2):
    RESULTS.clear()
    N, C = embed.shape
    H = W1.shape[1]
    K = W2.shape[1]
    CH = N // NCORES
    meta = _preprocess(np.asarray(edge_index), N)
    NW, NS, G, cumG, nslot = (meta['NW'], meta['NS'], meta['G'],
                              meta['cumG'], meta['nslot'])
    WT = NW * WIN
    BW = WT + 128
    cores = list(range(NCORES))

    # ---- NEFF 1
    nc1 = _build_neff1(N, C, H, CH)
    maps1 = []
    for c in range(NCORES):
        xt = np.ascontiguousarray(embed[c * CH:(c + 1) * CH, :].T)
        maps1.append({"xT": xt.astype(np.float32),
                      "W1": np.asarray(W1, np.float32),
                      "a1s": np.asarray(a_src1, np.float32)[:, None],
                      "a1d": np.asarray(a_dst1, np.float32)[:, None]})
    print("[kernel] NEFF1 built, running...", file=sys.stderr, flush=True)
    _res1 = run_bass_kernel_spmd(nc1, maps1, cores)
    RESULTS.append(_res1)
    r1 = _res1.results
    print("[kernel] NEFF1 done", file=sys.stderr, flush=True)

    T1 = np.zeros((N, 384), BF16)
    d1 = np.zeros((NCORES, 1, BW), np.float32)
    for c in range(NCORES):
        sl = slice(c * CH, (c + 1) * CH)
        T1[sl, 0:H] = r1[c]["hhi"].T
        T1[sl, H:2 * H] = r1[c]["hlo"].T
        T1[sl, 2 * H] = BF16(1.0)
        T1[sl, 2 * H + 1] = r1[c]["shi"][0]
        T1[sl, 2 * H + 2] = r1[c]["slo"][0]
        d1[c, 0, :CH] = r1[c]["d1o"][0]

    iota_np = np.tile(np.arange(128, dtype=np.float32), (128, 1)).astype(BF16)
    ones_np = np.ones((1, 128), BF16)

    # ---- NEFF 2
    nc2 = _build_edge_neff(N, CH, NW, NS, G, cumG, nslot, 1, H, K, None)
    maps2 = []
    for c in range(NCORES):
        m = {"T": T1, "dloc": d1[c], "iot": iota_np, "one1": ones_np,
             "brep": np.tile(np.asarray(b1, np.float32), (128, 1)),
             "W2": np.asarray(W2, np.float32),
             "a2s": np.asarray(a_src2, np.float32)[:, None],
             "a2d": np.asarray(a_dst2, np.float32)[:, None]}
        for s in range(NS):
            if nslot[s] == 0:
                continue
            m[f"gidx{s}"] = meta['cores'][c][0][s]
            m[f"dcol{s}"] = meta['cores'][c][1][s]
        maps2.append(m)
    print("[kernel] NEFF2 built, running...", file=sys.stderr, flush=True)
    _res2 = run_bass_kernel_spmd(nc2, maps2, cores)
    RESULTS.append(_res2)
    r2 = _res2.results
    print("[kernel] NEFF2 done", file=sys.stderr, flush=True)

    T2 = np.zeros((N, 256), BF16)
    d2 = np.zeros((NCORES, 1, BW), np.float32)
    for c in range(NCORES):
        sl = slice(c * CH, (c + 1) * CH)
        T2[sl, 0:K] = r2[c]["hhi"][:, :CH].T
        T2[sl, K:2 * K] = r2[c]["hlo"][:, :CH].T
        T2[sl, 2 * K] = BF16(1.0)
        T2[sl, 2 * K + 1] = r2[c]["shi"][0, :CH]
        T2[sl, 2 * K + 2] = r2[c]["slo"][0, :CH]
        d2[c, 0, :CH] = r2[c]["d2o"][0, :CH]

    # ---- NEFF 3
    nc3 = _build_edge_neff(N, CH, NW, NS, G, cumG, nslot, 2, K, None, None)
    maps3 = []
    for c in range(NCORES):
        m = {"T": T2, "dloc": d2[c], "iot": iota_np, "one1": ones_np,
             "brep": np.tile(np.asarray(b2, np.float32), (128, 1))}
        for s in range(NS):
            if nslot[s] == 0:
                continue
            m[f"gidx{s}"] = meta['cores'][c][0][s]
            m[f"dcol{s}"] = meta['cores'][c][1][s]
        maps3.append(m)
    print("[kernel] NEFF3 built, running...", file=sys.stderr, flush=True)
    _res3 = run_bass_kernel_spmd(nc3, maps3, cores)
    RESULTS.append(_res3)
    r3 = _res3.results
    print("[kernel] NEFF3 done", file=sys.stderr, flush=True)

    out = np.concatenate([r3[c]["out"] for c in range(NCORES)], axis=0)
    return out.astype(np.float32)



# revision 7
# speedup vs baseline: 1.6086x; 1.6086x over previous
"""2-layer GAT (PyG GATConv, heads=1) on 8 Trainium2 NeuronCores.

Strategy (dst-owner sharding per spec sharding_hint), 3 NEFF launches with
host doing only data movement/layout between them:

  NEFF#1: per-core h1 = embed_chunk @ W1 (f32), s1/d1 = h1 @ a_{src,dst}1.
  host:   assembles full h1 table, expands PER-EDGE tensors by fancy-index
          (pure data movement): gt1[slot] = bf16(h1[src_e]) plus per-edge
          s1[src_e], d1[dst_e] and dst-column ids. Everything is packed
          partition-major so the device streams it SEQUENTIALLY (no
          dma_gather / SWDGE descriptors - that was the 8ns/descriptor
          bottleneck of the previous version).
  NEFF#2: layer-1 edge phase per core:
            w_e = exp(leakyrelu(s_e + d_e))        (3 whole-layer ops)
            Ow_g[e, c] = w_e * 1[dstcol_e == c]    (ONE fused DVE/Pool op
                                                    per 128-edge group)
            psum_x[f, dst] += gt_g^T @ Ow_g        (TensorE, transposed so
            psum_z[1, dst] += ones^T @ Ow_g         softmax tail is cheap)
            x2T = relu(psum_x * (1/Z)_bcast + b1)  (per window)
          tail: h2 = W2^T x2T, s2/d2 = a2^T h2  -> host.
  NEFF#3: same machinery on [h2|1] (Z rides in the matmul via ones column),
          sigmoid, one output DMA [64, WT] per core; host transposes.

  Edges are grouped into 127-dst psum windows; group counts are maxed
  across cores so all 8 cores run one SPMD instruction stream.
"""
import sys

if '/opt/trn_rl_repo' not in sys.path:
    sys.path.insert(0, '/opt/trn_rl_repo')

import numpy as np
import ml_dtypes

from concourse import bacc, mybir
import concourse.tile as tile
from concourse.bass_utils import run_bass_kernel_spmd

BF16 = ml_dtypes.bfloat16
NCORES = 8
RESULTS = []  # BassKernelResults per NEFF launch (for test harness introspection)
WIN = 127          # dsts per psum window (col 127 = dummy slot for padding)
F32 = mybir.dt.float32
BF = mybir.dt.bfloat16
AF = mybir.ActivationFunctionType
OP = mybir.AluOpType


# ----------------------------------------------------------------- host pre
def _preprocess(edge_index, N):
    """Group edges by dst window, pad each (window) to a multiple of 128
    slots (counts maxed over cores for SPMD), and emit per-core slot->src,
    slot->dst, slot->dstcol arrays in partition-major [128, Gtot] layout."""
    CH = N // NCORES
    NW = -(-CH // WIN)
    src = np.concatenate([np.asarray(edge_index[0], np.int64),
                          np.arange(N, dtype=np.int64)])
    dst = np.concatenate([np.asarray(edge_index[1], np.int64),
                          np.arange(N, dtype=np.int64)])
    owner = dst // CH
    dl = dst - owner * CH

    percs = []
    cnt = np.zeros((NCORES, NW), np.int64)
    for c in range(NCORES):
        mc = owner == c
        cs, cd = src[mc], dl[mc]
        w = cd // WIN
        cnt[c] = np.bincount(w, minlength=NW)
        percs.append((cs, cd, w))
    G = -(-cnt.max(axis=0) // 128)          # groups per window, >=1
    base = np.zeros(NW + 1, np.int64)
    base[1:] = np.cumsum(128 * G)
    S = int(base[-1])
    Gtot = S // 128

    cores = []
    for c in range(NCORES):
        cs, cd, w = percs[c]
        order = np.argsort(w, kind='stable')
        cs, cd, w = cs[order], cd[order], w[order]
        cc = np.zeros(NW + 1, np.int64)
        cc[1:] = np.cumsum(cnt[c])
        rank = np.arange(len(cd)) - cc[w]
        slot = base[w] + rank
        srcslot = np.full(S, -1, np.int64)
        dstslot = np.full(S, -1, np.int64)
        colslot = np.full(S, 127.0, np.float32)
        srcslot[slot] = cs
        dstslot[slot] = cd + c * CH   # global dst id
        colslot[slot] = (cd - w * WIN).astype(np.float32)
        # partition-major: slot (g, p) -> [p, g]
        sidx = srcslot.reshape(Gtot, 128)            # [g, p] (slot-major)
        didx = dstslot.reshape(Gtot, 128)
        dcpm = np.ascontiguousarray(colslot.reshape(Gtot, 128).T.astype(np.float32))
        cores.append(dict(sidx=sidx, didx=didx, dc=dcpm))
    return dict(CH=CH, NW=NW, G=G, Gtot=Gtot, cores=cores)


def _expand(tbl_pad, idx_gp):
    """tbl_pad: [N+1, F] (last row zeros). idx_gp: [Gtot, 128] with -1 pads.
    Returns partition-major [128, Gtot, F] contiguous."""
    idx = np.where(idx_gp < 0, tbl_pad.shape[0] - 1, idx_gp)
    out = tbl_pad[idx]                       # [Gtot, 128, F]
    return np.ascontiguousarray(out.transpose(1, 0, 2))


def _expand1(vec_pad, idx_gp):
    """vec_pad: [N+1] (last = 0). Returns [128, Gtot] f32 contiguous."""
    idx = np.where(idx_gp < 0, vec_pad.shape[0] - 1, idx_gp)
    return np.ascontiguousarray(vec_pad[idx].T.astype(np.float32))


# ------------------------------------------------------------------ NEFF #1
def _build_neff1(N, C, H, CH):
    nc = bacc.Bacc(None, target_bir_lowering=False)
    xT = nc.declare_dram_parameter("xT", [C, CH], F32, isOutput=False)
    W1 = nc.declare_dram_parameter("W1", [C, H], F32, isOutput=False)
    a1s = nc.declare_dram_parameter("a1s", [H, 1], F32, isOutput=False)
    a1d = nc.declare_dram_parameter("a1d", [H, 1], F32, isOutput=False)
    hb = nc.declare_dram_parameter("hb", [H, CH], BF, isOutput=True)
    s1o = nc.declare_dram_parameter("s1o", [1, CH], F32, isOutput=True)
    d1o = nc.declare_dram_parameter("d1o", [1, CH], F32, isOutput=True)

    KT = -(-C // 128)
    with tile.TileContext(nc) as tc:
        with tc.tile_pool(name="cst", bufs=1) as cp, \
             tc.tile_pool(name="wk", bufs=3) as wp, \
             tc.tile_pool(name="ps", bufs=2, space="PSUM") as pp, \
             tc.tile_pool(name="ps1", bufs=2, space="PSUM") as pp1:
            xts, w1s = [], []
            for k in range(KT):
                kc = min(128, C - 128 * k)
                xt = cp.tile([kc, CH], F32, tag=f"xt{k}")
                nc.sync.dma_start(out=xt[:], in_=xT[128 * k:128 * k + kc, :])
                w1 = cp.tile([kc, H], F32, tag=f"w1{k}")
                nc.sync.dma_start(out=w1[:], in_=W1[128 * k:128 * k + kc, :])
                xts.append(xt)
                w1s.append(w1)
            asb = cp.tile([H, 1], F32, tag="a1s")
            nc.sync.dma_start(out=asb[:], in_=a1s[:])
            adb = cp.tile([H, 1], F32, tag="a1d")
            nc.sync.dma_start(out=adb[:], in_=a1d[:])
            h1T = cp.tile([H, CH], F32, tag="h1T")

            CW = 500
            for o in range(0, CH, CW):
                cw = min(CW, CH - o)
                ph = pp.tile([H, CW], F32, space="PSUM", tag="ph")
                for k in range(KT):
                    nc.tensor.matmul(out=ph[:, :cw], lhsT=w1s[k][:],
                                     rhs=xts[k][:, o:o + cw],
                                     start=(k == 0), stop=(k == KT - 1))
                nc.vector.tensor_copy(out=h1T[:, o:o + cw], in_=ph[:, :cw])
                hh = wp.tile([H, CW], BF, tag="hh")
                nc.scalar.activation(hh[:, :cw], ph[:, :cw], AF.Copy)
                nc.sync.dma_start(out=hb[:, o:o + cw], in_=hh[:, :cw])
            for o in range(0, CH, CW):
                cw = min(CW, CH - o)
                ps = pp1.tile([1, CW], F32, space="PSUM", tag="psv")
                nc.tensor.matmul(out=ps[:, :cw], lhsT=asb[:],
                                 rhs=h1T[:, o:o + cw], start=True, stop=True)
                sv = wp.tile([1, CW], F32, tag="sv")
                nc.vector.tensor_copy(out=sv[:, :cw], in_=ps[:, :cw])
                nc.sync.dma_start(out=s1o[:, o:o + cw], in_=sv[:, :cw])
                pd = pp1.tile([1, CW], F32, space="PSUM", tag="pdv")
                nc.tensor.matmul(out=pd[:, :cw], lhsT=adb[:],
                                 rhs=h1T[:, o:o + cw], start=True, stop=True)
                dv = wp.tile([1, CW], F32, tag="dv")
                nc.vector.tensor_copy(out=dv[:, :cw], in_=pd[:, :cw])
                nc.sync.dma_start(out=d1o[:, o:o + cw], in_=dv[:, :cw])
    nc.finalize()
    return nc


# --------------------------------------------------------- edge-phase NEFFs
def _build_edge_neff(layer, NW, G, Gtot, WT, H, K):
    """layer 1: FH=H msg cols, separate ones-matmul for Z; tail computes
    x2T=relu(out/Z+b1), h2=W2^T x2T, s2/d2.
    layer 2: LC=K+1 cols ([h2|1]); Z is row K of the same psum; tail is
    sigmoid; one output DMA."""
    FH = H if layer == 1 else K
    LC = FH if layer == 1 else FH + 1       # columns per slot in gt
    maxG = int(G.max())

    nc = bacc.Bacc(None, target_bir_lowering=False)
    gt = nc.declare_dram_parameter("gt", [128, Gtot * LC], BF, isOutput=False)
    dc = nc.declare_dram_parameter("dc", [128, Gtot], F32, isOutput=False)
    se = nc.declare_dram_parameter("se", [128, Gtot], F32, isOutput=False)
    de = nc.declare_dram_parameter("de", [128, Gtot], F32, isOutput=False)
    iot = nc.declare_dram_parameter("iot", [128, 128], BF, isOutput=False)
    onec = nc.declare_dram_parameter("onec", [128, 1], BF, isOutput=False)
    oner = nc.declare_dram_parameter("oner", [1, FH], F32, isOutput=False)
    bcol = nc.declare_dram_parameter("bcol", [FH, 1], F32, isOutput=False)
    if layer == 1:
        W2 = nc.declare_dram_parameter("W2", [H, K], F32, isOutput=False)
        a2s = nc.declare_dram_parameter("a2s", [K, 1], F32, isOutput=False)
        a2d = nc.declare_dram_parameter("a2d", [K, 1], F32, isOutput=False)
        h2o = nc.declare_dram_parameter("h2o", [K, WT], BF, isOutput=True)
        s2o = nc.declare_dram_parameter("s2o", [1, WT], F32, isOutput=True)
        d2o = nc.declare_dram_parameter("d2o", [1, WT], F32, isOutput=True)
    else:
        outp = nc.declare_dram_parameter("out", [K, WT], F32, isOutput=True)

    with tile.TileContext(nc) as tc:
        with tc.tile_pool(name="cst", bufs=1) as cp:
            iosb = cp.tile([128, 128], BF, tag="io")
            nc.sync.dma_start(out=iosb[:], in_=iot[:])
            onsb = cp.tile([128, 1], BF, tag="onc")
            nc.sync.dma_start(out=onsb[:], in_=onec[:])
            onr = cp.tile([1, FH], F32, tag="onr")
            nc.sync.dma_start(out=onr[:], in_=oner[:])
            bsb = cp.tile([FH, 1], F32, tag="bc")
            nc.sync.dma_start(out=bsb[:], in_=bcol[:])
            dcsb = cp.tile([128, Gtot], F32, tag="dc")
            nc.sync.dma_start(out=dcsb[:], in_=dc[:])
            wv = cp.tile([128, Gtot], F32, tag="wv")
            with tc.tile_pool(name="sd", bufs=1) as sdp:
                sesb = sdp.tile([128, Gtot], F32, tag="se")
                nc.sync.dma_start(out=sesb[:], in_=se[:])
                desb = sdp.tile([128, Gtot], F32, tag="de")
                nc.sync.dma_start(out=desb[:], in_=de[:])
                zv = sdp.tile([128, Gtot], F32, tag="zv")
                nc.vector.tensor_tensor(out=zv[:], in0=sesb[:], in1=desb[:],
                                        op=OP.add)
                lr = sdp.tile([128, Gtot], F32, tag="lr")
                nc.vector.scalar_tensor_tensor(out=lr[:], in0=zv[:],
                                               scalar=0.2, in1=zv[:],
                                               op0=OP.mult, op1=OP.max)
                nc.scalar.activation(wv[:], lr[:], AF.Exp)
            if layer == 1:
                x2T = cp.tile([128, WT], F32, tag="x2T")
                w2sb = cp.tile([H, K], F32, tag="w2")
                nc.sync.dma_start(out=w2sb[:], in_=W2[:])
                a2ssb = cp.tile([K, 1], F32, tag="a2s")
                nc.sync.dma_start(out=a2ssb[:], in_=a2s[:])
                a2dsb = cp.tile([K, 1], F32, tag="a2d")
                nc.sync.dma_start(out=a2dsb[:], in_=a2d[:])
            else:
                sgT = cp.tile([K, WT], F32, tag="sgT")

            with tc.tile_pool(name="gtp", bufs=3) as gtp, \
                 tc.tile_pool(name="owp", bufs=8) as owp, \
                 tc.tile_pool(name="wk", bufs=4) as wp, \
                 tc.tile_pool(name="psx", bufs=2, space="PSUM") as psxp, \
                 tc.tile_pool(name="psz", bufs=2, space="PSUM") as pszp, \
                 tc.tile_pool(name="pzr", bufs=2, space="PSUM") as pzrp:
                goff = 0
                for wdx in range(NW):
                    gw = int(G[wdx])
                    w0 = wdx * WIN
                    gtw = gtp.tile([128, maxG, LC], BF, tag="gt")
                    nc.sync.dma_start(
                        out=gtw[:, :gw, :],
                        in_=gt[:, goff * LC:(goff + gw) * LC])
                    if layer == 1:
                        psx = psxp.tile([FH, 128], F32, space="PSUM", tag="px")
                        psz = pszp.tile([1, 128], F32, space="PSUM", tag="pz")
                    else:
                        psx = psxp.tile([LC, 128], F32, space="PSUM", tag="px")
                    for j in range(gw):
                        g = goff + j
                        ow = owp.tile([128, 128], BF, tag="ow")
                        eng = nc.vector if (j % 2 == 0) else nc.gpsimd
                        eng.tensor_scalar(out=ow[:], in0=iosb[:],
                                          scalar1=dcsb[:, g:g + 1],
                                          scalar2=wv[:, g:g + 1],
                                          op0=OP.is_equal, op1=OP.mult)
                        nc.tensor.matmul(out=psx[:], lhsT=gtw[:, j, :],
                                         rhs=ow[:],
                                         start=(j == 0), stop=(j == gw - 1))
                        if layer == 1:
                            nc.tensor.matmul(out=psz[:], lhsT=onsb[:],
                                             rhs=ow[:],
                                             start=(j == 0), stop=(j == gw - 1))
                    # ---- window tail: normalize by Z, activation
                    zrow = psz[:] if layer == 1 else psx[FH:FH + 1, :]
                    z0 = wp.tile([1, 128], F32, tag="z0")
                    nc.vector.tensor_scalar(out=z0[:], in0=zrow,
                                            scalar1=1e-16, scalar2=None,
                                            op0=OP.add)
                    rz = wp.tile([1, 128], F32, tag="rz")
                    nc.vector.reciprocal(out=rz[:], in_=z0[:])
                    pzr = pzrp.tile([FH, 128], F32, space="PSUM", tag="pr")
                    nc.tensor.matmul(out=pzr[:], lhsT=onr[:], rhs=rz[:],
                                     start=True, stop=True)
                    zr = wp.tile([FH, 128], F32, tag="zr")
                    nc.scalar.activation(zr[:], pzr[:], AF.Copy)
                    xn = wp.tile([FH, WIN], F32, tag="xn")
                    nc.vector.tensor_tensor(out=xn[:], in0=psx[0:FH, 0:WIN],
                                            in1=zr[:, 0:WIN], op=OP.mult)
                    if layer == 1:
                        nc.scalar.activation(x2T[:, w0:w0 + WIN], xn[:],
                                             AF.Relu, bias=bsb[:])
                    else:
                        nc.scalar.activation(sgT[:, w0:w0 + WIN], xn[:],
                                             AF.Sigmoid, bias=bsb[:])
                    goff += gw

            if layer == 1:
                with tc.tile_pool(name="tl", bufs=3) as tp, \
                     tc.tile_pool(name="tc1", bufs=1) as tcp, \
                     tc.tile_pool(name="ph2", bufs=2, space="PSUM") as php, \
                     tc.tile_pool(name="psv", bufs=2, space="PSUM") as psp:
                    h2T = tcp.tile([K, WT], F32, tag="h2T")
                    CW = 512
                    for o in range(0, WT, CW):
                        cw = min(CW, WT - o)
                        ph = php.tile([K, CW], F32, space="PSUM", tag="ph")
                        nc.tensor.matmul(out=ph[:, :cw], lhsT=w2sb[:],
                                         rhs=x2T[:, o:o + cw],
                                         start=True, stop=True)
                        nc.vector.tensor_copy(out=h2T[:, o:o + cw],
                                              in_=ph[:, :cw])
                        hh = tp.tile([K, CW], BF, tag="hh")
                        nc.scalar.activation(hh[:, :cw], ph[:, :cw], AF.Copy)
                        nc.sync.dma_start(out=h2o[:, o:o + cw], in_=hh[:, :cw])
                    for o in range(0, WT, CW):
                        cw = min(CW, WT - o)
                        ps = psp.tile([1, CW], F32, space="PSUM", tag="ps2")
                        nc.tensor.matmul(out=ps[:, :cw], lhsT=a2ssb[:],
                                         rhs=h2T[:, o:o + cw],
                                         start=True, stop=True)
                        sv = tp.tile([1, CW], F32, tag="sv")
                        nc.vector.tensor_copy(out=sv[:, :cw], in_=ps[:, :cw])
                        nc.sync.dma_start(out=s2o[:, o:o + cw], in_=sv[:, :cw])
                        pd = psp.tile([1, CW], F32, space="PSUM", tag="pd")
                        nc.tensor.matmul(out=pd[:, :cw], lhsT=a2dsb[:],
                                         rhs=h2T[:, o:o + cw],
                                         start=True, stop=True)
                        dv = tp.tile([1, CW], F32, tag="dv")
                        nc.vector.tensor_copy(out=dv[:, :cw], in_=pd[:, :cw])
                        nc.sync.dma_start(out=d2o[:, o:o + cw], in_=dv[:, :cw])
            else:
                nc.sync.dma_start(out=outp[:], in_=sgT[:])
    nc.finalize()
    return nc


# ------------------------------------------------------------------- driver
def kernel(edge_index, embed, W1, a_src1, a_dst1, b1, W2, a_src2, a_dst2, b2):
    RESULTS.clear()
    N, C = embed.shape
    H = W1.shape[1]
    K = W2.shape[1]
    CH = N // NCORES
    meta = _preprocess(np.asarray(edge_index), N)
    NW, G, Gtot = meta['NW'], meta['G'], meta['Gtot']
    WT = NW * WIN
    cores = list(range(NCORES))

    # ---- NEFF 1
    nc1 = _build_neff1(N, C, H, CH)
    maps1 = []
    for c in range(NCORES):
        xt = np.ascontiguousarray(embed[c * CH:(c + 1) * CH, :].T)
        maps1.append({"xT": xt.astype(np.float32),
                      "W1": np.asarray(W1, np.float32),
                      "a1s": np.asarray(a_src1, np.float32)[:, None],
                      "a1d": np.asarray(a_dst1, np.float32)[:, None]})
    print("[kernel] NEFF1 built, running...", file=sys.stderr, flush=True)
    _res1 = run_bass_kernel_spmd(nc1, maps1, cores)
    RESULTS.append(_res1)
    r1 = _res1.results
    print("[kernel] NEFF1 done", file=sys.stderr, flush=True)

    # host: full h1 (bf16), s1, d1 tables with zero pad row
    h1p = np.zeros((N + 1, H), BF16)
    s1p = np.zeros(N + 1, np.float32)
    d1p = np.zeros(N + 1, np.float32)
    for c in range(NCORES):
        sl = slice(c * CH, (c + 1) * CH)
        h1p[sl] = r1[c]["hb"].T
        s1p[sl.start:sl.stop] = r1[c]["s1o"][0]
        d1p[sl.start:sl.stop] = r1[c]["d1o"][0]

    iota_np = np.tile(np.arange(128, dtype=np.float32), (128, 1)).astype(BF16)
    onec_np = np.ones((128, 1), BF16)

    # ---- NEFF 2
    nc2 = _build_edge_neff(1, NW, G, Gtot, WT, H, K)
    maps2 = []
    for c in range(NCORES):
        m = meta['cores'][c]
        gt1 = _expand(h1p, m['sidx']).reshape(128, Gtot * H)
        maps2.append({
            "gt": gt1, "dc": m['dc'],
            "se": _expand1(s1p, m['sidx']),
            "de": _expand1(d1p, m['didx']),
            "iot": iota_np, "onec": onec_np,
            "oner": np.ones((1, H), np.float32),
            "bcol": np.asarray(b1, np.float32)[:, None],
            "W2": np.asarray(W2, np.float32),
            "a2s": np.asarray(a_src2, np.float32)[:, None],
            "a2d": np.asarray(a_dst2, np.float32)[:, None]})
    print("[kernel] NEFF2 built, running...", file=sys.stderr, flush=True)
    _res2 = run_bass_kernel_spmd(nc2, maps2, cores)
    RESULTS.append(_res2)
    r2 = _res2.results
    print("[kernel] NEFF2 done", file=sys.stderr, flush=True)

    # host: full [h2|1] (bf16), s2, d2 tables
    h2p = np.zeros((N + 1, K + 1), BF16)
    s2p = np.zeros(N + 1, np.float32)
    d2p = np.zeros(N + 1, np.float32)
    for c in range(NCORES):
        sl = slice(c * CH, (c + 1) * CH)
        h2p[sl, :K] = r2[c]["h2o"][:, :CH].T
        h2p[sl, K] = BF16(1.0)
        s2p[sl.start:sl.stop] = r2[c]["s2o"][0, :CH]
        d2p[sl.start:sl.stop] = r2[c]["d2o"][0, :CH]

    # ---- NEFF 3
    nc3 = _build_edge_neff(2, NW, G, Gtot, WT, H, K)
    maps3 = []
    for c in range(NCORES):
        m = meta['cores'][c]
        gt2 = _expand(h2p, m['sidx']).reshape(128, Gtot * (K + 1))
        maps3.append({
            "gt": gt2, "dc": m['dc'],
            "se": _expand1(s2p, m['sidx']),
            "de": _expand1(d2p, m['didx']),
            "iot": iota_np, "onec": onec_np,
            "oner": np.ones((1, K), np.float32),
            "bcol": np.asarray(b2, np.float32)[:, None]})
    print("[kernel] NEFF3 built, running...", file=sys.stderr, flush=True)
    _res3 = run_bass_kernel_spmd(nc3, maps3, cores)
    RESULTS.append(_res3)
    r3 = _res3.results
    print("[kernel] NEFF3 done", file=sys.stderr, flush=True)

    out = np.concatenate(
        [r3[c]["out"][:, :CH].T for c in range(NCORES)], axis=0)
    return np.ascontiguousarray(out).astype(np.float32)


# revision 10
# speedup vs baseline: 87.3522x; 54.3044x over previous
"""2-layer GAT (PyG GATConv, heads=1) on 8 Trainium2 NeuronCores.

Strategy (dst-owner sharding per spec sharding_hint), 3 NEFF launches with
host doing only data movement/layout between them:

  NEFF#1: per-core h1 = embed_chunk @ W1 (f32), s1/d1 = h1 @ a_{src,dst}1.
  host:   assembles full h1 table, expands PER-EDGE tensors by fancy-index
          (pure data movement): gt1[slot] = bf16(h1[src_e]) plus per-edge
          s1[src_e], d1[dst_e] and dst-column ids. Everything is packed
          partition-major so the device streams it SEQUENTIALLY (no
          dma_gather / SWDGE descriptors - that was the 8ns/descriptor
          bottleneck of the previous version).
  NEFF#2: layer-1 edge phase per core:
            w_e = exp(leakyrelu(s_e + d_e))          (3 whole-layer ops)
            msg = [h|1] * w  via ONE stride-0-broadcast tensor_tensor per
                  127-dst window (per-partition-scalar ops cost ~1-2us on
                  HW regardless of width - avoid them in hot loops!)
            psum[dst, {f,Z}] += O_g^T @ msg_g        (ONE matmul per group;
                  O = host-shipped raw one-hot; Z rides in the ones column)
            tail/window: rz=1/Z (per-dst = per-partition), relu via
                  max(x*rz, -b1) trick, transpose -> x2T; bias restored in
                  h2 = W2^T x2T + W2^T b1; s2/d2 = a2^T h2 -> host.
  NEFF#3: same on [h2|1], sigmoid tail -> [128, NW*64]; host unshuffles.

  Edges are grouped into 127-dst psum windows; group counts are maxed
  across cores so all 8 cores run one SPMD instruction stream.
"""
import sys

if '/opt/trn_rl_repo' not in sys.path:
    sys.path.insert(0, '/opt/trn_rl_repo')

import numpy as np
import ml_dtypes

from concourse import bacc, mybir
import concourse.tile as tile
from concourse.bass_utils import run_bass_kernel_spmd

BF16 = ml_dtypes.bfloat16
NCORES = 8
RESULTS = []  # BassKernelResults per NEFF launch (for test harness introspection)
WIN = 127          # dsts per psum window (col 127 = dummy slot for padding)
F32 = mybir.dt.float32
BF = mybir.dt.bfloat16
AF = mybir.ActivationFunctionType
OP = mybir.AluOpType


# ----------------------------------------------------------------- host pre
def _preprocess(edge_index, N):
    """Group edges by dst window, pad each (window) to a multiple of 128
    slots (counts maxed over cores for SPMD), and emit per-core slot->src,
    slot->dst, slot->dstcol arrays in partition-major [128, Gtot] layout."""
    CH = N // NCORES
    NW = -(-CH // WIN)
    src = np.concatenate([np.asarray(edge_index[0], np.int64),
                          np.arange(N, dtype=np.int64)])
    dst = np.concatenate([np.asarray(edge_index[1], np.int64),
                          np.arange(N, dtype=np.int64)])
    owner = dst // CH
    dl = dst - owner * CH

    percs = []
    cnt = np.zeros((NCORES, NW), np.int64)
    for c in range(NCORES):
        mc = owner == c
        cs, cd = src[mc], dl[mc]
        w = cd // WIN
        cnt[c] = np.bincount(w, minlength=NW)
        percs.append((cs, cd, w))
    G = -(-cnt.max(axis=0) // 128)          # groups per window, >=1
    base = np.zeros(NW + 1, np.int64)
    base[1:] = np.cumsum(128 * G)
    S = int(base[-1])
    Gtot = S // 128

    cores = []
    for c in range(NCORES):
        cs, cd, w = percs[c]
        order = np.argsort(w, kind='stable')
        cs, cd, w = cs[order], cd[order], w[order]
        cc = np.zeros(NW + 1, np.int64)
        cc[1:] = np.cumsum(cnt[c])
        rank = np.arange(len(cd)) - cc[w]
        slot = base[w] + rank
        srcslot = np.full(S, -1, np.int64)
        dstslot = np.full(S, -1, np.int64)
        colslot = np.full(S, -1, np.int64)    # -1 pad -> all-zero one-hot row
        srcslot[slot] = cs
        dstslot[slot] = cd + c * CH   # global dst id
        colslot[slot] = cd - w * WIN
        # partition-major: slot (g, p) -> [p, g]
        sidx = srcslot.reshape(Gtot, 128)            # [g, p] (slot-major)
        didx = dstslot.reshape(Gtot, 128)
        # one-hot rows O[p, g, c] = 1[colslot==c]; pad rows all-zero
        eye = np.zeros((129, 128), BF16)
        eye[np.arange(1, 128), np.arange(127)] = BF16(1.0)
        O = eye[colslot.reshape(Gtot, 128) + 1]      # [g, p, 128]
        O = np.ascontiguousarray(O.transpose(1, 0, 2)).reshape(128, Gtot * 128)
        cores.append(dict(sidx=sidx, didx=didx, O=O))
    return dict(CH=CH, NW=NW, G=G, Gtot=Gtot, cores=cores)


def _expand(tbl_pad, idx_gp):
    """tbl_pad: [N+1, F] (last row zeros). idx_gp: [Gtot, 128] with -1 pads.
    Returns partition-major [128, Gtot, F] contiguous."""
    idx = np.where(idx_gp < 0, tbl_pad.shape[0] - 1, idx_gp)
    out = tbl_pad[idx]                       # [Gtot, 128, F]
    return np.ascontiguousarray(out.transpose(1, 0, 2))


def _expand1(vec_pad, idx_gp):
    """vec_pad: [N+1] (last = 0). Returns [128, Gtot] f32 contiguous."""
    idx = np.where(idx_gp < 0, vec_pad.shape[0] - 1, idx_gp)
    return np.ascontiguousarray(vec_pad[idx].T.astype(np.float32))


# ------------------------------------------------------------------ NEFF #1
def _build_neff1(N, C, H, CH):
    nc = bacc.Bacc(None, target_bir_lowering=False)
    xT = nc.declare_dram_parameter("xT", [C, CH], F32, isOutput=False)
    W1 = nc.declare_dram_parameter("W1", [C, H], F32, isOutput=False)
    a1s = nc.declare_dram_parameter("a1s", [H, 1], F32, isOutput=False)
    a1d = nc.declare_dram_parameter("a1d", [H, 1], F32, isOutput=False)
    hb = nc.declare_dram_parameter("hb", [H, CH], BF, isOutput=True)
    s1o = nc.declare_dram_parameter("s1o", [1, CH], F32, isOutput=True)
    d1o = nc.declare_dram_parameter("d1o", [1, CH], F32, isOutput=True)

    KT = -(-C // 128)
    with tile.TileContext(nc) as tc:
        with tc.tile_pool(name="cst", bufs=1) as cp, \
             tc.tile_pool(name="wk", bufs=3) as wp, \
             tc.tile_pool(name="ps", bufs=2, space="PSUM") as pp, \
             tc.tile_pool(name="ps1", bufs=2, space="PSUM") as pp1:
            xts, w1s = [], []
            for k in range(KT):
                kc = min(128, C - 128 * k)
                xt = cp.tile([kc, CH], F32, tag=f"xt{k}")
                nc.sync.dma_start(out=xt[:], in_=xT[128 * k:128 * k + kc, :])
                w1 = cp.tile([kc, H], F32, tag=f"w1{k}")
                nc.sync.dma_start(out=w1[:], in_=W1[128 * k:128 * k + kc, :])
                xts.append(xt)
                w1s.append(w1)
            asb = cp.tile([H, 1], F32, tag="a1s")
            nc.sync.dma_start(out=asb[:], in_=a1s[:])
            adb = cp.tile([H, 1], F32, tag="a1d")
            nc.sync.dma_start(out=adb[:], in_=a1d[:])
            h1T = cp.tile([H, CH], F32, tag="h1T")

            CW = 500
            for o in range(0, CH, CW):
                cw = min(CW, CH - o)
                ph = pp.tile([H, CW], F32, space="PSUM", tag="ph")
                for k in range(KT):
                    nc.tensor.matmul(out=ph[:, :cw], lhsT=w1s[k][:],
                                     rhs=xts[k][:, o:o + cw],
                                     start=(k == 0), stop=(k == KT - 1))
                nc.vector.tensor_copy(out=h1T[:, o:o + cw], in_=ph[:, :cw])
                hh = wp.tile([H, CW], BF, tag="hh")
                nc.scalar.activation(hh[:, :cw], ph[:, :cw], AF.Copy)
                nc.sync.dma_start(out=hb[:, o:o + cw], in_=hh[:, :cw])
            for o in range(0, CH, CW):
                cw = min(CW, CH - o)
                ps = pp1.tile([1, CW], F32, space="PSUM", tag="psv")
                nc.tensor.matmul(out=ps[:, :cw], lhsT=asb[:],
                                 rhs=h1T[:, o:o + cw], start=True, stop=True)
                sv = wp.tile([1, CW], F32, tag="sv")
                nc.vector.tensor_copy(out=sv[:, :cw], in_=ps[:, :cw])
                nc.sync.dma_start(out=s1o[:, o:o + cw], in_=sv[:, :cw])
                pd = pp1.tile([1, CW], F32, space="PSUM", tag="pdv")
                nc.tensor.matmul(out=pd[:, :cw], lhsT=adb[:],
                                 rhs=h1T[:, o:o + cw], start=True, stop=True)
                dv = wp.tile([1, CW], F32, tag="dv")
                nc.vector.tensor_copy(out=dv[:, :cw], in_=pd[:, :cw])
                nc.sync.dma_start(out=d1o[:, o:o + cw], in_=dv[:, :cw])
    nc.finalize()
    return nc


# --------------------------------------------------------- edge-phase NEFFs
def _build_edge_neff(layer, NW, G, Gtot, WT, H, K):
    """Per window: ONE broadcast tensor_tensor folds w into the [h|1] message
    rows (msg = gt * w), then one matmul per 128-edge group accumulates
    psum[dst, {f,Z}] += O_g^T @ msg_g  (O = raw one-hot, host-shipped).
    Tail: rz = 1/Z (per-partition = per-dst), then
      layer 1: xm = max(psum*rz, -b1) = relu(out+b1)-b1, transpose -> x2T;
               h2 = W2^T x2T + (W2^T b1), s2/d2 = a2^T h2.
      layer 2: sig = Sigmoid(psum*rz + b2) -> [128, NW*K] (host unshuffles).
    """
    FH = H if layer == 1 else K
    LC = FH + 1                          # gt cols per slot: [h | 1]
    maxG = int(G.max())

    nc = bacc.Bacc(None, target_bir_lowering=False)
    gt = nc.declare_dram_parameter("gt", [128, Gtot * LC], BF, isOutput=False)
    Od = nc.declare_dram_parameter("O", [128, Gtot * 128], BF, isOutput=False)
    se = nc.declare_dram_parameter("se", [128, Gtot], F32, isOutput=False)
    de = nc.declare_dram_parameter("de", [128, Gtot], F32, isOutput=False)
    brep = nc.declare_dram_parameter("brep", [128, FH], F32, isOutput=False)
    if layer == 1:
        W2 = nc.declare_dram_parameter("W2", [H, K], F32, isOutput=False)
        a2s = nc.declare_dram_parameter("a2s", [K, 1], F32, isOutput=False)
        a2d = nc.declare_dram_parameter("a2d", [K, 1], F32, isOutput=False)
        c2 = nc.declare_dram_parameter("c2", [K, 1], F32, isOutput=False)
        h2o = nc.declare_dram_parameter("h2o", [K, WT], BF, isOutput=True)
        s2o = nc.declare_dram_parameter("s2o", [1, WT], F32, isOutput=True)
        d2o = nc.declare_dram_parameter("d2o", [1, WT], F32, isOutput=True)
    else:
        outp = nc.declare_dram_parameter("out", [128, NW * K], F32,
                                         isOutput=True)

    from concourse.masks import make_identity
    with tile.TileContext(nc) as tc:
        with tc.tile_pool(name="cst", bufs=1) as cp:
            bsb = cp.tile([128, FH], F32, tag="br")
            nc.sync.dma_start(out=bsb[:], in_=brep[:])
            wv = cp.tile([128, Gtot], BF, tag="wv")
            with tc.tile_pool(name="sd", bufs=1) as sdp:
                sesb = sdp.tile([128, Gtot], F32, tag="se")
                nc.sync.dma_start(out=sesb[:], in_=se[:])
                desb = sdp.tile([128, Gtot], F32, tag="de")
                nc.sync.dma_start(out=desb[:], in_=de[:])
                zv = sdp.tile([128, Gtot], F32, tag="zv")
                nc.vector.tensor_tensor(out=zv[:], in0=sesb[:], in1=desb[:],
                                        op=OP.add)
                lr = sdp.tile([128, Gtot], F32, tag="lr")
                nc.vector.scalar_tensor_tensor(out=lr[:], in0=zv[:],
                                               scalar=0.2, in1=zv[:],
                                               op0=OP.mult, op1=OP.max)
                nc.scalar.activation(wv[:], lr[:], AF.Exp)
            if layer == 1:
                idn = cp.tile([128, 128], F32, tag="idn")
                make_identity(nc, idn[:])
                x2T = cp.tile([128, WT], F32, tag="x2T")
                w2sb = cp.tile([H, K], F32, tag="w2")
                nc.sync.dma_start(out=w2sb[:], in_=W2[:])
                a2ssb = cp.tile([K, 1], F32, tag="a2s")
                nc.sync.dma_start(out=a2ssb[:], in_=a2s[:])
                a2dsb = cp.tile([K, 1], F32, tag="a2d")
                nc.sync.dma_start(out=a2dsb[:], in_=a2d[:])
                c2sb = cp.tile([K, 1], F32, tag="c2")
                nc.sync.dma_start(out=c2sb[:], in_=c2[:])
            else:
                sgT = cp.tile([128, NW * K], F32, tag="sgT")

            with tc.tile_pool(name="gtp", bufs=3) as gtp, \
                 tc.tile_pool(name="otp", bufs=3) as otp, \
                 tc.tile_pool(name="msp", bufs=3) as msp, \
                 tc.tile_pool(name="wk", bufs=4) as wp, \
                 tc.tile_pool(name="psx", bufs=2, space="PSUM") as psxp, \
                 tc.tile_pool(name="ptr", bufs=2, space="PSUM") as ptrp:
                goff = 0
                for wdx in range(NW):
                    gw = int(G[wdx])
                    w0 = wdx * WIN
                    gtw = gtp.tile([128, maxG, LC], BF, tag="gt")
                    nc.sync.dma_start(
                        out=gtw[:, :gw, :],
                        in_=gt[:, goff * LC:(goff + gw) * LC])
                    ot = otp.tile([128, maxG, 128], BF, tag="ot")
                    nc.sync.dma_start(
                        out=ot[:, :gw, :],
                        in_=Od[:, goff * 128:(goff + gw) * 128])
                    msgw = msp.tile([128, maxG, LC], BF, tag="ms")
                    feng = nc.vector if (wdx % 2 == 0) else nc.gpsimd
                    feng.tensor_tensor(
                        out=msgw[:, :gw, :], in0=gtw[:, :gw, :],
                        in1=wv[:, goff:goff + gw].to_broadcast((128, gw, LC)),
                        op=OP.mult)
                    psx = psxp.tile([128, LC], F32, space="PSUM", tag="px")
                    for j in range(gw):
                        nc.tensor.matmul(out=psx[:], lhsT=ot[:, j, :],
                                         rhs=msgw[:, j, :],
                                         start=(j == 0), stop=(j == gw - 1))
                    # ---- window tail
                    rz = wp.tile([128, 1], F32, tag="rz")
                    nc.vector.reciprocal(out=rz[:], in_=psx[:, FH:FH + 1])
                    xm = wp.tile([128, FH], F32, tag="xm")
                    nc.vector.scalar_tensor_tensor(
                        out=xm[:], in0=psx[:, 0:FH], scalar=rz[:],
                        in1=bsb[:], op0=OP.mult,
                        op1=(OP.max if layer == 1 else OP.add))
                    if layer == 1:
                        pt = ptrp.tile([128, 128], F32, space="PSUM", tag="pt")
                        nc.tensor.transpose(pt[:], xm[:], idn[:])
                        nc.scalar.activation(x2T[:, w0:w0 + WIN],
                                             pt[:, 0:WIN], AF.Copy)
                    else:
                        nc.scalar.activation(sgT[:, wdx * K:(wdx + 1) * K],
                                             xm[:], AF.Sigmoid)
                    goff += gw

            if layer == 1:
                with tc.tile_pool(name="tl", bufs=3) as tp, \
                     tc.tile_pool(name="tc1", bufs=1) as tcp, \
                     tc.tile_pool(name="ph2", bufs=2, space="PSUM") as php, \
                     tc.tile_pool(name="psv", bufs=2, space="PSUM") as psp:
                    h2T = tcp.tile([K, WT], F32, tag="h2T")
                    CW = 512
                    for o in range(0, WT, CW):
                        cw = min(CW, WT - o)
                        ph = php.tile([K, CW], F32, space="PSUM", tag="ph")
                        nc.tensor.matmul(out=ph[:, :cw], lhsT=w2sb[:],
                                         rhs=x2T[:, o:o + cw],
                                         start=True, stop=True)
                        # h2 = W2^T xm^T + c2  (c2 = W2^T b1 restores bias)
                        nc.vector.tensor_scalar(
                            out=h2T[:, o:o + cw], in0=ph[:, :cw],
                            scalar1=c2sb[:], scalar2=None, op0=OP.add)
                        hh = tp.tile([K, CW], BF, tag="hh")
                        nc.scalar.activation(hh[:, :cw], h2T[:, o:o + cw],
                                             AF.Copy)
                        nc.sync.dma_start(out=h2o[:, o:o + cw], in_=hh[:, :cw])
                    for o in range(0, WT, CW):
                        cw = min(CW, WT - o)
                        ps = psp.tile([1, CW], F32, space="PSUM", tag="ps2")
                        nc.tensor.matmul(out=ps[:, :cw], lhsT=a2ssb[:],
                                         rhs=h2T[:, o:o + cw],
                                         start=True, stop=True)
                        sv = tp.tile([1, CW], F32, tag="sv")
                        nc.vector.tensor_copy(out=sv[:, :cw], in_=ps[:, :cw])
                        nc.sync.dma_start(out=s2o[:, o:o + cw], in_=sv[:, :cw])
                        pd = psp.tile([1, CW], F32, space="PSUM", tag="pd")
                        nc.tensor.matmul(out=pd[:, :cw], lhsT=a2dsb[:],
                                         rhs=h2T[:, o:o + cw],
                                         start=True, stop=True)
                        dv = tp.tile([1, CW], F32, tag="dv")
                        nc.vector.tensor_copy(out=dv[:, :cw], in_=pd[:, :cw])
                        nc.sync.dma_start(out=d2o[:, o:o + cw], in_=dv[:, :cw])
            else:
                nc.sync.dma_start(out=outp[:], in_=sgT[:])
    nc.finalize()
    return nc


# ------------------------------------------------------------------- driver
def kernel(edge_index, embed, W1, a_src1, a_dst1, b1, W2, a_src2, a_dst2, b2):
    RESULTS.clear()
    N, C = embed.shape
    H = W1.shape[1]
    K = W2.shape[1]
    CH = N // NCORES
    meta = _preprocess(np.asarray(edge_index), N)
    NW, G, Gtot = meta['NW'], meta['G'], meta['Gtot']
    WT = NW * WIN
    cores = list(range(NCORES))

    # ---- NEFF 1
    nc1 = _build_neff1(N, C, H, CH)
    maps1 = []
    for c in range(NCORES):
        xt = np.ascontiguousarray(embed[c * CH:(c + 1) * CH, :].T)
        maps1.append({"xT": xt.astype(np.float32),
                      "W1": np.asarray(W1, np.float32),
                      "a1s": np.asarray(a_src1, np.float32)[:, None],
                      "a1d": np.asarray(a_dst1, np.float32)[:, None]})
    print("[kernel] NEFF1 built, running...", file=sys.stderr, flush=True)
    _res1 = run_bass_kernel_spmd(nc1, maps1, cores)
    RESULTS.append(_res1)
    r1 = _res1.results
    print("[kernel] NEFF1 done", file=sys.stderr, flush=True)

    # host: full [h1|1] (bf16), s1, d1 tables with zero pad row
    h1e = np.zeros((N + 1, H + 1), BF16)
    s1p = np.zeros(N + 1, np.float32)
    d1p = np.zeros(N + 1, np.float32)
    for c in range(NCORES):
        sl = slice(c * CH, (c + 1) * CH)
        h1e[sl, :H] = r1[c]["hb"].T
        h1e[sl, H] = BF16(1.0)
        s1p[sl.start:sl.stop] = r1[c]["s1o"][0]
        d1p[sl.start:sl.stop] = r1[c]["d1o"][0]

    # ---- NEFF 2
    nc2 = _build_edge_neff(1, NW, G, Gtot, WT, H, K)
    b1f = np.asarray(b1, np.float32)
    c2v = (np.asarray(W2, np.float32).T @ b1f)[:, None]
    maps2 = []
    for c in range(NCORES):
        m = meta['cores'][c]
        gt1 = _expand(h1e, m['sidx']).reshape(128, Gtot * (H + 1))
        maps2.append({
            "gt": gt1, "O": m['O'],
            "se": _expand1(s1p, m['sidx']),
            "de": _expand1(d1p, m['didx']),
            "brep": np.tile(-b1f, (128, 1)),
            "W2": np.asarray(W2, np.float32),
            "a2s": np.asarray(a_src2, np.float32)[:, None],
            "a2d": np.asarray(a_dst2, np.float32)[:, None],
            "c2": c2v})
    print("[kernel] NEFF2 built, running...", file=sys.stderr, flush=True)
    _res2 = run_bass_kernel_spmd(nc2, maps2, cores)
    RESULTS.append(_res2)
    r2 = _res2.results
    print("[kernel] NEFF2 done", file=sys.stderr, flush=True)

    # host: full [h2|1] (bf16), s2, d2 tables
    h2p = np.zeros((N + 1, K + 1), BF16)
    s2p = np.zeros(N + 1, np.float32)
    d2p = np.zeros(N + 1, np.float32)
    for c in range(NCORES):
        sl = slice(c * CH, (c + 1) * CH)
        h2p[sl, :K] = r2[c]["h2o"][:, :CH].T
        h2p[sl, K] = BF16(1.0)
        s2p[sl.start:sl.stop] = r2[c]["s2o"][0, :CH]
        d2p[sl.start:sl.stop] = r2[c]["d2o"][0, :CH]

    # ---- NEFF 3
    nc3 = _build_edge_neff(2, NW, G, Gtot, WT, H, K)
    maps3 = []
    for c in range(NCORES):
        m = meta['cores'][c]
        gt2 = _expand(h2p, m['sidx']).reshape(128, Gtot * (K + 1))
        maps3.append({
            "gt": gt2, "O": m['O'],
            "se": _expand1(s2p, m['sidx']),
            "de": _expand1(d2p, m['didx']),
            "brep": np.tile(np.asarray(b2, np.float32), (128, 1))})
    print("[kernel] NEFF3 built, running...", file=sys.stderr, flush=True)
    _res3 = run_bass_kernel_spmd(nc3, maps3, cores)
    RESULTS.append(_res3)
    r3 = _res3.results
    print("[kernel] NEFF3 done", file=sys.stderr, flush=True)

    # unshuffle: r3[c]["out"][p, w*K:(w+1)*K] is node c*CH + w*WIN + p (p<WIN)
    out = np.empty((N, K), np.float32)
    for c in range(NCORES):
        sg = r3[c]["out"].reshape(128, NW, K).transpose(1, 0, 2)  # [w, p, K]
        out[c * CH:(c + 1) * CH] = sg[:, :WIN, :].reshape(NW * WIN, K)[:CH]
    return out
